# revision 15
# baseline (speedup 1.0000x reference)
"""ConcatAttention Trainium2 kernel (8-core data-parallel over batch).

Computes, per batch row b:
    scores[b, l] = sum_h v[h] * tanh(q_proj[b, h] + (key_val[l, b] @ Wk)[h])
    out[b, 0, :] = softmax(scores[b, :])

Device-side per core (B_shard = 4 batch rows):
  - main matmul  kpT[h, l] = Wk^T @ keyT   (float32r, K=512 via 4 PSUM-accum chunks)
  - ACT fuses    energy = tanh(kpT + q_projT[h])  (per-partition bias)
  - v-dot        scores[1, l] = v^T @ energy      (M=1 matmuls, PSUM accum over h)
  - ACT exp with fused accum_out chunk sums; normalization happens on host
    (device ships exp(s - U_b) and the per-chunk sums).

Host side shards/reshapes: key_val is laid out [b][h_in][L] per core so the
device streams fully contiguous tiles, the tiny q_proj = query @ Wq is
precomputed on host, and the final softmax divide runs in float64 on host.
"""

import os
import sys

for _p in ("/opt/trn_rl_repo", os.path.expanduser("~/trn_rl_repo")):
    if os.path.isdir(_p) and _p not in sys.path:
        sys.path.insert(0, _p)

import numpy as np

L, B, H = 4096, 32, 512
NCORES = 8
BS = B // NCORES          # batch rows per core
P = 128
CI = H // P               # input-feature chunks (contraction)
CH = H // P               # output-feature chunks
LC = 512                  # l-tile (matmul moving free dim)
NLC = L // LC
QRT = L // 4              # key DMA granularity: [128, QRT] = 512 KiB
WARMUP_MM = 8             # dense dummy matmuls to heat the PE HAM clock gate
                          # (8 x 427ns cold = 3.4us contiguous busy, exactly
                          # one HAM SHORT window -> PE warm before real work)

# Per-b compute chunk plans (start, width). b0 ramps up with narrow chunks
# so compute can begin as soon as the first small key DMAs land.
CHUNKS_B0 = [(0, 128), (128, 384)] + [(512 * k, 512) for k in range(1, NLC)]
CHUNKS_BN = [(512 * k, 512) for k in range(NLC)]
NCH_MAX = max(len(CHUNKS_B0), len(CHUNKS_BN))

_CACHE = {}


def _build_nc():
    import concourse.bacc as bacc
    import concourse.mybir as mybir
    import concourse.tile as tile

    f32 = mybir.dt.float32
    f32r = mybir.dt.float32r
    Act = mybir.ActivationFunctionType

    nc = bacc.Bacc("TRN2", target_bir_lowering=False)

    keyT = nc.dram_tensor("keyT", [BS, CI, P, L], f32r, kind="ExternalInput")
    wk = nc.dram_tensor("wk", [P, CI, H], f32r, kind="ExternalInput")
    qpT = nc.dram_tensor("qpT", [P, CH, BS], f32, kind="ExternalInput")
    vT = nc.dram_tensor("vT", [P, CH], f32r, kind="ExternalInput")
    # -U_b: softmax shift per batch row (host-derived safe bound near the
    # row max; softmax is invariant to the exact value)
    negu = nc.dram_tensor("negu", [1, BS], f32, kind="ExternalInput")
    out = nc.dram_tensor("out", [BS, L], f32, kind="ExternalOutput")
    sums_out = nc.dram_tensor("sums_out", [BS, NCH_MAX], f32,
                              kind="ExternalOutput")

    with tile.TileContext(nc) as tc:
        with tc.tile_pool(name="singles", bufs=1) as singles, \
             tc.tile_pool(name="ktp", bufs=8) as ktp, \
             tc.tile_pool(name="kts_small", bufs=8) as ktsp, \
             tc.tile_pool(name="enp", bufs=8) as enp, \
             tc.tile_pool(name="scrp", bufs=2) as scrp, \
             tc.tile_pool(name="kpp", bufs=6, space="PSUM") as kpp, \
             tc.tile_pool(name="scp", bufs=2, space="PSUM") as scp:

            def load_kt(b, plan, tiles=None, pos=0):
                """plan: list of l-slice widths; each slice is one joint DMA
                carrying all CI feature chunks."""
                if tiles is None:
                    tiles = []
                for w in plan:
                    t = ktp.tile([P, CI, QRT], f32r, tag="kt")
                    nc.sync.dma_start(
                        t[:, :, :w],
                        keyT[b, :, :, pos:pos + w].rearrange("c p l -> p c l"))
                    tiles.append((pos, w, t))
                    pos += w
                return tiles

            def load_kt_perci(b, pos, w, tiles, queues=None):
                """One slice as CI separate tiles/DMAs so the first matmuls
                can start as soon as their own chunk lands."""
                grp = []
                for ci in range(CI):
                    t = ktsp.tile([P, w], f32r, tag="kts",
                                  name=f"kts_{b}_{pos}_{ci}")
                    q = queues[ci] if queues else nc.sync
                    q.dma_start(t, keyT[b, ci, :, pos:pos + w])
                    grp.append(t)
                tiles.append((pos, w, grp))
                return tiles

            def kt_slice(tiles, ci, l0, w):
                for pos, tw, t in tiles:
                    if pos <= l0 and l0 + w <= pos + tw:
                        if isinstance(t, list):
                            return t[ci][:, l0 - pos:l0 - pos + w]
                        return t[:, ci, l0 - pos:l0 - pos + w]
                raise AssertionError("no tile covers slice")

            # ---- head loads: the first narrow key chunk rides the two
            # otherwise-idle queues while wk streams on sync, so the first
            # real matmul can start ~3us after DMA becomes possible ----
            wk_sb = singles.tile([P, CI, H], f32r, tag="wk")
            qpT_sb = singles.tile([P, CH, BS], f32, tag="qpT")
            vT_sb = singles.tile([P, CH], f32r, tag="vT")
            negu_sb = singles.tile([1, BS], f32, tag="negu")
            kts = []
            load_kt_perci(0, 0, 128, kts,
                          queues=[nc.gpsimd, nc.gpsimd, nc.scalar, nc.scalar])
            nc.sync.dma_start(wk_sb[:, 0, :], wk[:, 0, :])
            nc.gpsimd.dma_start(wk_sb[:, 1, :], wk[:, 1, :])
            nc.sync.dma_start(wk_sb[:, 2, :], wk[:, 2, :])
            nc.scalar.dma_start(wk_sb[:, 3, :], wk[:, 3, :])
            nc.gpsimd.dma_start(qpT_sb, qpT[:, :, :])
            nc.scalar.dma_start(vT_sb, vT[:, :])
            nc.gpsimd.dma_start(negu_sb, negu[:, :])
            load_kt_perci(0, 128, 384, kts)
            load_kt_perci(0, 512, 512, kts)
            load_kt(0, [LC, LC, QRT, QRT], tiles=kts, pos=2 * LC)

            # ---- PE warmup: dense dummy matmuls on zeros while the first
            # key tiles stream in, so the HAM clock gate reaches 2.4 GHz
            # as early as possible; plus a tiny activation to pull the
            # ~2.7us ACT table load into the head shadow ----
            wu = singles.tile([P, LC], f32, tag="warmup")
            nc.vector.memset(wu, 0.0)
            wur = wu[:, :].bitcast(f32r)
            trash_act = singles.tile([1, 1], f32, tag="trash_act")
            nc.scalar.activation(trash_act, wu[0:1, 0:1], Act.Tanh)
            for g in range(WARMUP_MM):
                wps = kpp.tile([P, LC], f32, tag="kp")
                nc.tensor.matmul(wps, wur[:, 0:P], wur, start=True, stop=True)

            def emit_vdot(b, w, ens):
                sc = scp.tile([1, LC], mybir.dt.float32, tag="sc")
                for ch in range(CH):
                    nc.tensor.matmul(sc[:, :w], vT_sb[:, ch:ch + 1], ens[ch],
                                     start=(ch == 0), stop=(ch == CH - 1))
                return sc

            HL = L // 2
            for b in range(BS):
                # Chunked softmax with a fixed host-supplied shift U_b:
                # exp each chunk straight out of PSUM as it completes, with
                # the chunk sum fused into the same ACT instruction.
                chunks = CHUNKS_B0 if b == 0 else CHUNKS_BN
                scores = scrp.tile([1, L], f32, tag="scores")
                csums = singles.tile([1, NCH_MAX], f32, tag=f"csums_{b}")

                def finish_chunk(idx, pens, chunks=chunks, scores=scores,
                                 csums=csums, b=b):
                    s0, w = chunks[idx]
                    sc = emit_vdot(b, w, pens)
                    sl = scores[:, s0:s0 + w]
                    nc.scalar.activation(sl, sc[:, :w], Act.Exp,
                                         bias=negu_sb[:, b:b + 1],
                                         accum_out=csums[:, idx:idx + 1])
                    # ship each finished half right away (no normalize on
                    # device; the host divides by the summed csums)
                    if s0 + w in (HL, L):
                        h2 = (s0 + w) // HL - 1
                        nc.sync.dma_start(
                            out[b:b + 1, h2 * HL:(h2 + 1) * HL],
                            scores[:, h2 * HL:(h2 + 1) * HL])

                pending = None  # (idx, ens) awaiting v-dot emission
                for idx, (s0, w) in enumerate(chunks):
                    ens = []
                    for ch in range(CH):
                        ps = kpp.tile([P, LC], f32, tag="kp")
                        for ci in range(CI):
                            nc.tensor.matmul(
                                ps[:, :w],
                                wk_sb[:, ci, ch * P:(ch + 1) * P],
                                kt_slice(kts, ci, s0, w),
                                start=(ci == 0), stop=(ci == CI - 1))
                        en = enp.tile([P, w], f32r, tag="en",
                                      name=f"en_{b}_{s0}_{ch}")
                        nc.scalar.activation(en, ps[:, :w], Act.Tanh,
                                             bias=qpT_sb[:, ch, b:b + 1])
                        ens.append(en)
                    # software-pipeline: emit previous chunk's v-dot after this
                    # chunk's main matmuls so PE never waits on ACT.
                    if pending is not None:
                        finish_chunk(*pending)
                    pending = (idx, ens)
                # prefetch next b's key tiles before the final chunk
                if b + 1 < BS:
                    next_kts = load_kt(b + 1, [QRT] * 4)
                finish_chunk(*pending)
                nc.sync.dma_start(sums_out[b:b + 1, :len(chunks)],
                                  csums[:, :len(chunks)])
                if b + 1 < BS:
                    kts = next_kts

    nc.compile()
    return nc


def _get_nc():
    if "nc" not in _CACHE:
        _CACHE["nc"] = _build_nc()
    return _CACHE["nc"]


def _prep_inputs(query, key_val, W, v):
    """Host-side shard prep: returns list of 8 per-core input dicts."""
    query = np.asarray(query, dtype=np.float32)
    key_val = np.asarray(key_val, dtype=np.float32)
    W = np.asarray(W, dtype=np.float32)
    v = np.asarray(v, dtype=np.float32)

    q_proj = (query.astype(np.float64) @ W[:H].astype(np.float64)).astype(np.float32)
    wk_tiled = np.ascontiguousarray(
        W[H:].reshape(CI, P, H).transpose(1, 0, 2))          # [P, CI, H]
    vT_tiled = np.ascontiguousarray(v.reshape(CH, P).T)      # [P, CH]

    # Sample a handful of exact scores per row to place the softmax shift U_b
    # near the row max (any U within ~80 of the max is numerically exact).
    ls = np.linspace(0, L - 1, 64).astype(np.int64)
    kp_s = np.einsum("lbi,ih->lbh", key_val[ls].astype(np.float64),
                     W[H:].astype(np.float64))               # (64, B, H)
    sc_s = np.einsum("h,lbh->bl", v.astype(np.float64),
                     np.tanh(q_proj.astype(np.float64)[None] + kp_s))
    U = sc_s.max(axis=1) + 40.0                              # (B,)

    in_maps = []
    for c in range(NCORES):
        b0 = c * BS
        # key_val[l, b, i] -> [b, ci, p(i), l]
        kt = np.ascontiguousarray(
            key_val[:, b0:b0 + BS, :].transpose(1, 2, 0)
            .reshape(BS, CI, P, L))
        qpT_tiled = np.ascontiguousarray(
            q_proj[b0:b0 + BS].T.reshape(CH, P, BS).transpose(1, 0, 2))
        in_maps.append({
            "keyT": kt,
            "wk": wk_tiled,
            "qpT": qpT_tiled,
            "vT": vT_tiled,
            "negu": np.ascontiguousarray(
                -U[b0:b0 + BS].astype(np.float32).reshape(1, BS)),
        })
    return in_maps


def _run(inputs, trace=False, **trace_kwargs):
    from concourse.bass_utils import run_bass_kernel_spmd

    nc = _get_nc()
    in_maps = _prep_inputs(**inputs)
    res = run_bass_kernel_spmd(
        nc, in_maps, core_ids=list(range(NCORES)), trace=trace, **trace_kwargs)
    expd = np.concatenate(
        [np.asarray(r["out"], dtype=np.float32) for r in res.results],
        axis=0).astype(np.float64)                            # (B, L)
    nch = [len(CHUNKS_B0)] + [len(CHUNKS_BN)] * (BS - 1)      # per local b
    S = np.concatenate(
        [np.array([np.asarray(r["sums_out"], dtype=np.float64)[b, :nch[b]].sum()
                   for b in range(BS)])
         for r in res.results])                               # (B,)
    out = (expd / S[:, None]).astype(np.float32)
    return out.reshape(B, 1, L), res


def kernel(**inputs):
    out, _ = _run(inputs, trace=False)
    return out


# revision 16
# speedup vs baseline: 1.0336x; 1.0336x over previous
"""ConcatAttention Trainium2 kernel (8-core data-parallel over batch).

Computes, per batch row b:
    scores[b, l] = sum_h v[h] * tanh(q_proj[b, h] + (key_val[l, b] @ Wk)[h])
    out[b, 0, :] = softmax(scores[b, :])

Device-side per core (B_shard = 4 batch rows):
  - main matmul  kpT[h, l] = Wk^T @ keyT   (float32r, K=512 via 4 PSUM-accum chunks)
  - ACT fuses    energy = tanh(kpT + q_projT[h])  (per-partition bias)
  - v-dot        scores[1, l] = v^T @ energy      (M=1 matmuls, PSUM accum over h)
  - ACT exp with fused accum_out chunk sums; normalization happens on host
    (device ships exp(s - U_b) and the per-chunk sums).

Host side shards/reshapes: key_val is laid out [b][h_in][L] per core so the
device streams fully contiguous tiles, the tiny q_proj = query @ Wq is
precomputed on host, and the final softmax divide runs in float64 on host.
"""

import os
import sys

for _p in ("/opt/trn_rl_repo", os.path.expanduser("~/trn_rl_repo")):
    if os.path.isdir(_p) and _p not in sys.path:
        sys.path.insert(0, _p)

import numpy as np

L, B, H = 4096, 32, 512
NCORES = 8
BS = B // NCORES          # batch rows per core
P = 128
CI = H // P               # input-feature chunks (contraction)
CH = H // P               # output-feature chunks
LC = 512                  # l-tile (matmul moving free dim)
NLC = L // LC
QRT = L // 4              # key DMA granularity: [128, QRT] = 512 KiB
WARMUP_MM = 8             # dense dummy matmuls to heat the PE HAM clock gate
                          # (8 x 427ns cold = 3.4us contiguous busy, exactly
                          # one HAM SHORT window -> PE warm before real work)

# Per-b compute chunk plans (start, width). b0 ramps up with narrow chunks
# so compute can begin as soon as the first small key DMAs land.
CHUNKS_B0 = [(0, 128), (128, 384)] + [(512 * k, 512) for k in range(1, NLC)]
CHUNKS_BN = [(512 * k, 512) for k in range(NLC)]
NCH_MAX = max(len(CHUNKS_B0), len(CHUNKS_BN))

_CACHE = {}


def _build_nc():
    import concourse.bacc as bacc
    import concourse.mybir as mybir
    import concourse.tile as tile

    f32 = mybir.dt.float32
    f32r = mybir.dt.float32r
    Act = mybir.ActivationFunctionType

    nc = bacc.Bacc("TRN2", target_bir_lowering=False)

    keyT = nc.dram_tensor("keyT", [BS, CI, P, L], f32r, kind="ExternalInput")
    wk = nc.dram_tensor("wk", [P, CI, H], f32r, kind="ExternalInput")
    qpT = nc.dram_tensor("qpT", [P, CH, BS], f32, kind="ExternalInput")
    vT = nc.dram_tensor("vT", [P, CH], f32r, kind="ExternalInput")
    # -U_b: softmax shift per batch row (host-derived safe bound near the
    # row max; softmax is invariant to the exact value)
    negu = nc.dram_tensor("negu", [1, BS], f32, kind="ExternalInput")
    out = nc.dram_tensor("out", [BS, L], f32, kind="ExternalOutput")
    sums_out = nc.dram_tensor("sums_out", [BS, NCH_MAX], f32,
                              kind="ExternalOutput")

    with tile.TileContext(nc) as tc:
        with tc.tile_pool(name="singles", bufs=1) as singles, \
             tc.tile_pool(name="ktp", bufs=8) as ktp, \
             tc.tile_pool(name="kts_small", bufs=8) as ktsp, \
             tc.tile_pool(name="enp", bufs=8) as enp, \
             tc.tile_pool(name="scrp", bufs=2) as scrp, \
             tc.tile_pool(name="kpp", bufs=6, space="PSUM") as kpp, \
             tc.tile_pool(name="scp", bufs=2, space="PSUM") as scp:

            def load_kt(b, plan, tiles=None, pos=0):
                """plan: list of l-slice widths; each slice is one joint DMA
                carrying all CI feature chunks."""
                if tiles is None:
                    tiles = []
                for w in plan:
                    t = ktp.tile([P, CI, QRT], f32r, tag="kt")
                    nc.sync.dma_start(
                        t[:, :, :w],
                        keyT[b, :, :, pos:pos + w].rearrange("c p l -> p c l"))
                    tiles.append((pos, w, t))
                    pos += w
                return tiles

            def load_kt_perci(b, pos, w, tiles, queues=None):
                """One slice as CI separate tiles/DMAs so the first matmuls
                can start as soon as their own chunk lands."""
                grp = []
                for ci in range(CI):
                    t = ktsp.tile([P, w], f32r, tag="kts",
                                  name=f"kts_{b}_{pos}_{ci}")
                    q = queues[ci] if queues else nc.sync
                    q.dma_start(t, keyT[b, ci, :, pos:pos + w])
                    grp.append(t)
                tiles.append((pos, w, grp))
                return tiles

            def kt_slice(tiles, ci, l0, w):
                for pos, tw, t in tiles:
                    if pos <= l0 and l0 + w <= pos + tw:
                        if isinstance(t, list):
                            return t[ci][:, l0 - pos:l0 - pos + w]
                        return t[:, ci, l0 - pos:l0 - pos + w]
                raise AssertionError("no tile covers slice")

            # ---- head loads: the first narrow key chunk rides the two
            # otherwise-idle queues while wk streams on sync, so the first
            # real matmul can start ~3us after DMA becomes possible ----
            wk_sb = singles.tile([P, CI, H], f32r, tag="wk")
            qpT_sb = singles.tile([P, CH, BS], f32, tag="qpT")
            vT_sb = singles.tile([P, CH], f32r, tag="vT")
            negu_sb = singles.tile([1, BS], f32, tag="negu")
            kts = []
            load_kt_perci(0, 0, 128, kts,
                          queues=[nc.gpsimd, nc.gpsimd, nc.scalar, nc.scalar])
            for ci in range(CI):
                nc.sync.dma_start(wk_sb[:, ci, :], wk[:, ci, :])
            load_kt_perci(0, 128, 384, kts)
            nc.gpsimd.dma_start(qpT_sb, qpT[:, :, :])
            nc.scalar.dma_start(vT_sb, vT[:, :])
            nc.gpsimd.dma_start(negu_sb, negu[:, :])
            load_kt_perci(0, 512, 512, kts)
            load_kt(0, [LC, LC, QRT, QRT], tiles=kts, pos=2 * LC)

            # ---- PE warmup: dense dummy matmuls on zeros while the first
            # key tiles stream in, so the HAM clock gate reaches 2.4 GHz
            # as early as possible; plus a tiny activation to pull the
            # ~2.7us ACT table load into the head shadow ----
            wu = singles.tile([P, LC], f32, tag="warmup")
            nc.vector.memset(wu, 0.0)
            wur = wu[:, :].bitcast(f32r)
            trash_act = singles.tile([1, 1], f32, tag="trash_act")
            nc.scalar.activation(trash_act, wu[0:1, 0:1], Act.Tanh)
            for g in range(WARMUP_MM):
                wps = kpp.tile([P, LC], f32, tag="kp")
                nc.tensor.matmul(wps, wur[:, 0:P], wur, start=True, stop=True)

            def emit_vdot(b, w, ens):
                sc = scp.tile([1, LC], mybir.dt.float32, tag="sc")
                for ch in range(CH):
                    nc.tensor.matmul(sc[:, :w], vT_sb[:, ch:ch + 1], ens[ch],
                                     start=(ch == 0), stop=(ch == CH - 1))
                return sc

            HL = L // 2
            for b in range(BS):
                # Chunked softmax with a fixed host-supplied shift U_b:
                # exp each chunk straight out of PSUM as it completes, with
                # the chunk sum fused into the same ACT instruction.
                chunks = CHUNKS_B0 if b == 0 else CHUNKS_BN
                scores = scrp.tile([1, L], f32, tag="scores")
                csums = singles.tile([1, NCH_MAX], f32, tag=f"csums_{b}")

                def finish_chunk(idx, pens, chunks=chunks, scores=scores,
                                 csums=csums, b=b):
                    s0, w = chunks[idx]
                    sc = emit_vdot(b, w, pens)
                    sl = scores[:, s0:s0 + w]
                    nc.scalar.activation(sl, sc[:, :w], Act.Exp,
                                         bias=negu_sb[:, b:b + 1],
                                         accum_out=csums[:, idx:idx + 1])
                    # ship each finished half right away (no normalize on
                    # device; the host divides by the summed csums)
                    if s0 + w in (HL, L):
                        h2 = (s0 + w) // HL - 1
                        nc.sync.dma_start(
                            out[b:b + 1, h2 * HL:(h2 + 1) * HL],
                            scores[:, h2 * HL:(h2 + 1) * HL])

                pending = None  # (idx, ens) awaiting v-dot emission
                for idx, (s0, w) in enumerate(chunks):
                    ens = []
                    for ch in range(CH):
                        ps = kpp.tile([P, LC], f32, tag="kp")
                        for ci in range(CI):
                            nc.tensor.matmul(
                                ps[:, :w],
                                wk_sb[:, ci, ch * P:(ch + 1) * P],
                                kt_slice(kts, ci, s0, w),
                                start=(ci == 0), stop=(ci == CI - 1))
                        en = enp.tile([P, w], f32r, tag="en",
                                      name=f"en_{b}_{s0}_{ch}")
                        nc.scalar.activation(en, ps[:, :w], Act.Tanh,
                                             bias=qpT_sb[:, ch, b:b + 1])
                        ens.append(en)
                    # software-pipeline: emit previous chunk's v-dot after this
                    # chunk's main matmuls so PE never waits on ACT.
                    if pending is not None:
                        finish_chunk(*pending)
                    pending = (idx, ens)
                # prefetch next b's key tiles before the final chunk
                if b + 1 < BS:
                    next_kts = load_kt(b + 1, [QRT] * 4)
                finish_chunk(*pending)
                nc.sync.dma_start(sums_out[b:b + 1, :len(chunks)],
                                  csums[:, :len(chunks)])
                if b + 1 < BS:
                    kts = next_kts

    nc.compile()
    return nc


def _get_nc():
    if "nc" not in _CACHE:
        _CACHE["nc"] = _build_nc()
    return _CACHE["nc"]


def _prep_inputs(query, key_val, W, v):
    """Host-side shard prep: returns list of 8 per-core input dicts."""
    query = np.asarray(query, dtype=np.float32)
    key_val = np.asarray(key_val, dtype=np.float32)
    W = np.asarray(W, dtype=np.float32)
    v = np.asarray(v, dtype=np.float32)

    q_proj = (query.astype(np.float64) @ W[:H].astype(np.float64)).astype(np.float32)
    wk_tiled = np.ascontiguousarray(
        W[H:].reshape(CI, P, H).transpose(1, 0, 2))          # [P, CI, H]
    vT_tiled = np.ascontiguousarray(v.reshape(CH, P).T)      # [P, CH]

    # Sample a handful of exact scores per row to place the softmax shift U_b
    # near the row max (any U within ~80 of the max is numerically exact).
    ls = np.linspace(0, L - 1, 64).astype(np.int64)
    kp_s = np.einsum("lbi,ih->lbh", key_val[ls].astype(np.float64),
                     W[H:].astype(np.float64))               # (64, B, H)
    sc_s = np.einsum("h,lbh->bl", v.astype(np.float64),
                     np.tanh(q_proj.astype(np.float64)[None] + kp_s))
    U = sc_s.max(axis=1) + 40.0                              # (B,)

    in_maps = []
    for c in range(NCORES):
        b0 = c * BS
        # key_val[l, b, i] -> [b, ci, p(i), l]
        kt = np.ascontiguousarray(
            key_val[:, b0:b0 + BS, :].transpose(1, 2, 0)
            .reshape(BS, CI, P, L))
        qpT_tiled = np.ascontiguousarray(
            q_proj[b0:b0 + BS].T.reshape(CH, P, BS).transpose(1, 0, 2))
        in_maps.append({
            "keyT": kt,
            "wk": wk_tiled,
            "qpT": qpT_tiled,
            "vT": vT_tiled,
            "negu": np.ascontiguousarray(
                -U[b0:b0 + BS].astype(np.float32).reshape(1, BS)),
        })
    return in_maps


def _run(inputs, trace=False, **trace_kwargs):
    from concourse.bass_utils import run_bass_kernel_spmd

    nc = _get_nc()
    in_maps = _prep_inputs(**inputs)
    res = run_bass_kernel_spmd(
        nc, in_maps, core_ids=list(range(NCORES)), trace=trace, **trace_kwargs)
    expd = np.concatenate(
        [np.asarray(r["out"], dtype=np.float32) for r in res.results],
        axis=0).astype(np.float64)                            # (B, L)
    nch = [len(CHUNKS_B0)] + [len(CHUNKS_BN)] * (BS - 1)      # per local b
    S = np.concatenate(
        [np.array([np.asarray(r["sums_out"], dtype=np.float64)[b, :nch[b]].sum()
                   for b in range(BS)])
         for r in res.results])                               # (B,)
    out = (expd / S[:, None]).astype(np.float32)
    return out.reshape(B, 1, L), res


def kernel(**inputs):
    out, _ = _run(inputs, trace=False)
    return out


# revision 18
# speedup vs baseline: 1.0361x; 1.0024x over previous
"""ConcatAttention Trainium2 kernel (8-core data-parallel over batch).

Computes, per batch row b:
    scores[b, l] = sum_h v[h] * tanh(q_proj[b, h] + (key_val[l, b] @ Wk)[h])
    out[b, 0, :] = softmax(scores[b, :])

Device-side per core (B_shard = 4 batch rows):
  - main matmul  kpT[h, l] = Wk^T @ keyT   (float32r, K=512 via 4 PSUM-accum chunks)
  - ACT fuses    energy = tanh(kpT + q_projT[h])  (per-partition bias)
  - v-dot        scores[1, l] = v^T @ energy      (M=1 matmuls, PSUM accum over h)
  - ACT exp with fused accum_out chunk sums; normalization happens on host
    (device ships exp(s - U_b) and the per-chunk sums).

Host side shards/reshapes: key_val is laid out [b][h_in][L] per core so the
device streams fully contiguous tiles, the tiny q_proj = query @ Wq is
precomputed on host, and the final softmax divide runs in float64 on host.
"""

import os
import sys

for _p in ("/opt/trn_rl_repo", os.path.expanduser("~/trn_rl_repo")):
    if os.path.isdir(_p) and _p not in sys.path:
        sys.path.insert(0, _p)

import numpy as np

L, B, H = 4096, 32, 512
NCORES = 8
BS = B // NCORES          # batch rows per core
P = 128
CI = H // P               # input-feature chunks (contraction)
CH = H // P               # output-feature chunks
LC = 512                  # l-tile (matmul moving free dim)
NLC = L // LC
QRT = L // 4              # key DMA granularity: [128, QRT] = 512 KiB
WARMUP_MM = 16            # dense dummy matmuls to heat the PE HAM clock gate
                          # (the HAM needs one fully-busy ALIGNED 3.41us
                          # window; a ~2x-window contiguous burst guarantees
                          # that regardless of phase)

# Per-b compute chunk plans (start, width).
CHUNKS_B0 = [(512 * k, 512) for k in range(NLC)]
CHUNKS_BN = [(512 * k, 512) for k in range(NLC)]
NCH_MAX = max(len(CHUNKS_B0), len(CHUNKS_BN))

_CACHE = {}


def _build_nc():
    import concourse.bacc as bacc
    import concourse.mybir as mybir
    import concourse.tile as tile

    f32 = mybir.dt.float32
    f32r = mybir.dt.float32r
    Act = mybir.ActivationFunctionType

    nc = bacc.Bacc("TRN2", target_bir_lowering=False)

    keyT = nc.dram_tensor("keyT", [BS, CI, P, L], f32r, kind="ExternalInput")
    wk = nc.dram_tensor("wk", [P, CI, H], f32r, kind="ExternalInput")
    qpT = nc.dram_tensor("qpT", [P, CH, BS], f32, kind="ExternalInput")
    vT = nc.dram_tensor("vT", [P, CH], f32r, kind="ExternalInput")
    # -U_b: softmax shift per batch row (host-derived safe bound near the
    # row max; softmax is invariant to the exact value)
    negu = nc.dram_tensor("negu", [1, BS], f32, kind="ExternalInput")
    out = nc.dram_tensor("out", [BS, L], f32, kind="ExternalOutput")
    sums_out = nc.dram_tensor("sums_out", [BS, NCH_MAX], f32,
                              kind="ExternalOutput")

    with tile.TileContext(nc) as tc:
        with tc.tile_pool(name="singles", bufs=1) as singles, \
             tc.tile_pool(name="ktp", bufs=8) as ktp, \
             tc.tile_pool(name="kts_small", bufs=8) as ktsp, \
             tc.tile_pool(name="enp", bufs=8) as enp, \
             tc.tile_pool(name="scrp", bufs=2) as scrp, \
             tc.tile_pool(name="kpp", bufs=6, space="PSUM") as kpp, \
             tc.tile_pool(name="scp", bufs=2, space="PSUM") as scp:

            def load_kt(b, plan, tiles=None, pos=0):
                """plan: list of l-slice widths; each slice is one joint DMA
                carrying all CI feature chunks."""
                if tiles is None:
                    tiles = []
                for w in plan:
                    t = ktp.tile([P, CI, QRT], f32r, tag="kt")
                    nc.sync.dma_start(
                        t[:, :, :w],
                        keyT[b, :, :, pos:pos + w].rearrange("c p l -> p c l"))
                    tiles.append((pos, w, t))
                    pos += w
                return tiles

            def load_kt_perci(b, pos, w, tiles, queues=None):
                """One slice as CI separate tiles/DMAs so the first matmuls
                can start as soon as their own chunk lands."""
                grp = []
                for ci in range(CI):
                    t = ktsp.tile([P, w], f32r, tag="kts",
                                  name=f"kts_{b}_{pos}_{ci}")
                    q = queues[ci] if queues else nc.sync
                    q.dma_start(t, keyT[b, ci, :, pos:pos + w])
                    grp.append(t)
                tiles.append((pos, w, grp))
                return tiles

            def kt_slice(tiles, ci, l0, w):
                for pos, tw, t in tiles:
                    if pos <= l0 and l0 + w <= pos + tw:
                        if isinstance(t, list):
                            return t[ci][:, l0 - pos:l0 - pos + w]
                        return t[:, ci, l0 - pos:l0 - pos + w]
                raise AssertionError("no tile covers slice")

            # ---- head loads: the first narrow key chunk rides the two
            # otherwise-idle queues while wk streams on sync, so the first
            # real matmul can start ~3us after DMA becomes possible ----
            wk_sb = singles.tile([P, CI, H], f32r, tag="wk")
            qpT_sb = singles.tile([P, CH, BS], f32, tag="qpT")
            vT_sb = singles.tile([P, CH], f32r, tag="vT")
            negu_sb = singles.tile([1, BS], f32, tag="negu")
            kts = []
            # first key chunk rides the (idle, HWDGE) scalar queue while the
            # weights stream on sync; tiny constants go to gpsimd (SWDGE,
            # slow first-call — nothing latency-critical there)
            load_kt_perci(0, 0, 512, kts,
                          queues=[nc.scalar, nc.scalar, nc.scalar, nc.scalar])
            for ci in range(CI):
                nc.sync.dma_start(wk_sb[:, ci, :], wk[:, ci, :])
            nc.gpsimd.dma_start(qpT_sb, qpT[:, :, :])
            nc.gpsimd.dma_start(vT_sb, vT[:, :])
            nc.gpsimd.dma_start(negu_sb, negu[:, :])
            load_kt_perci(0, 512, 512, kts)
            load_kt(0, [LC, LC, QRT, QRT], tiles=kts, pos=2 * LC)

            # ---- PE warmup: dense dummy matmuls on zeros while the first
            # key tiles stream in, so the HAM clock gate reaches 2.4 GHz
            # as early as possible; plus a tiny activation to pull the
            # ~2.7us ACT table load into the head shadow ----
            wu = singles.tile([P, LC], f32, tag="warmup")
            nc.vector.memset(wu, 0.0)
            wur = wu[:, :].bitcast(f32r)
            trash_act = singles.tile([1, 1], f32, tag="trash_act")
            nc.scalar.activation(trash_act, wu[0:1, 0:1], Act.Tanh)
            for g in range(WARMUP_MM):
                wps = kpp.tile([P, LC], f32, tag="kp")
                nc.tensor.matmul(wps, wur[:, 0:P], wur, start=True, stop=True)

            def emit_vdot(b, w, ens):
                sc = scp.tile([1, LC], mybir.dt.float32, tag="sc")
                for ch in range(CH):
                    nc.tensor.matmul(sc[:, :w], vT_sb[:, ch:ch + 1], ens[ch],
                                     start=(ch == 0), stop=(ch == CH - 1))
                return sc

            HL = L // 2
            for b in range(BS):
                # Chunked softmax with a fixed host-supplied shift U_b:
                # exp each chunk straight out of PSUM as it completes, with
                # the chunk sum fused into the same ACT instruction.
                chunks = CHUNKS_B0 if b == 0 else CHUNKS_BN
                scores = scrp.tile([1, L], f32, tag="scores")
                csums = singles.tile([1, NCH_MAX], f32, tag=f"csums_{b}")

                def finish_chunk(idx, pens, chunks=chunks, scores=scores,
                                 csums=csums, b=b):
                    s0, w = chunks[idx]
                    sc = emit_vdot(b, w, pens)
                    sl = scores[:, s0:s0 + w]
                    nc.scalar.activation(sl, sc[:, :w], Act.Exp,
                                         bias=negu_sb[:, b:b + 1],
                                         accum_out=csums[:, idx:idx + 1])
                    # ship each finished half right away (no normalize on
                    # device; the host divides by the summed csums)
                    if s0 + w in (HL, L):
                        h2 = (s0 + w) // HL - 1
                        nc.sync.dma_start(
                            out[b:b + 1, h2 * HL:(h2 + 1) * HL],
                            scores[:, h2 * HL:(h2 + 1) * HL])

                pending = None  # (idx, ens) awaiting v-dot emission
                for idx, (s0, w) in enumerate(chunks):
                    ens = []
                    for ch in range(CH):
                        ps = kpp.tile([P, LC], f32, tag="kp")
                        for ci in range(CI):
                            nc.tensor.matmul(
                                ps[:, :w],
                                wk_sb[:, ci, ch * P:(ch + 1) * P],
                                kt_slice(kts, ci, s0, w),
                                start=(ci == 0), stop=(ci == CI - 1))
                        en = enp.tile([P, w], f32r, tag="en",
                                      name=f"en_{b}_{s0}_{ch}")
                        nc.scalar.activation(en, ps[:, :w], Act.Tanh,
                                             bias=qpT_sb[:, ch, b:b + 1])
                        ens.append(en)
                    # software-pipeline: emit previous chunk's v-dot after this
                    # chunk's main matmuls so PE never waits on ACT.
                    if pending is not None:
                        finish_chunk(*pending)
                    pending = (idx, ens)
                # prefetch next b's key tiles before the final chunk
                if b + 1 < BS:
                    next_kts = load_kt(b + 1, [QRT] * 4)
                finish_chunk(*pending)
                nc.sync.dma_start(sums_out[b:b + 1, :len(chunks)],
                                  csums[:, :len(chunks)])
                if b + 1 < BS:
                    kts = next_kts

    nc.compile()
    return nc


def _get_nc():
    if "nc" not in _CACHE:
        _CACHE["nc"] = _build_nc()
    return _CACHE["nc"]


def _prep_inputs(query, key_val, W, v):
    """Host-side shard prep: returns list of 8 per-core input dicts."""
    query = np.asarray(query, dtype=np.float32)
    key_val = np.asarray(key_val, dtype=np.float32)
    W = np.asarray(W, dtype=np.float32)
    v = np.asarray(v, dtype=np.float32)

    q_proj = (query.astype(np.float64) @ W[:H].astype(np.float64)).astype(np.float32)
    wk_tiled = np.ascontiguousarray(
        W[H:].reshape(CI, P, H).transpose(1, 0, 2))          # [P, CI, H]
    vT_tiled = np.ascontiguousarray(v.reshape(CH, P).T)      # [P, CH]

    # Sample a handful of exact scores per row to place the softmax shift U_b
    # near the row max (any U within ~80 of the max is numerically exact).
    ls = np.linspace(0, L - 1, 64).astype(np.int64)
    kp_s = np.einsum("lbi,ih->lbh", key_val[ls].astype(np.float64),
                     W[H:].astype(np.float64))               # (64, B, H)
    sc_s = np.einsum("h,lbh->bl", v.astype(np.float64),
                     np.tanh(q_proj.astype(np.float64)[None] + kp_s))
    U = sc_s.max(axis=1) + 40.0                              # (B,)

    in_maps = []
    for c in range(NCORES):
        b0 = c * BS
        # key_val[l, b, i] -> [b, ci, p(i), l]
        kt = np.ascontiguousarray(
            key_val[:, b0:b0 + BS, :].transpose(1, 2, 0)
            .reshape(BS, CI, P, L))
        qpT_tiled = np.ascontiguousarray(
            q_proj[b0:b0 + BS].T.reshape(CH, P, BS).transpose(1, 0, 2))
        in_maps.append({
            "keyT": kt,
            "wk": wk_tiled,
            "qpT": qpT_tiled,
            "vT": vT_tiled,
            "negu": np.ascontiguousarray(
                -U[b0:b0 + BS].astype(np.float32).reshape(1, BS)),
        })
    return in_maps


def _run(inputs, trace=False, **trace_kwargs):
    from concourse.bass_utils import run_bass_kernel_spmd

    nc = _get_nc()
    in_maps = _prep_inputs(**inputs)
    res = run_bass_kernel_spmd(
        nc, in_maps, core_ids=list(range(NCORES)), trace=trace, **trace_kwargs)
    expd = np.concatenate(
        [np.asarray(r["out"], dtype=np.float32) for r in res.results],
        axis=0).astype(np.float64)                            # (B, L)
    nch = [len(CHUNKS_B0)] + [len(CHUNKS_BN)] * (BS - 1)      # per local b
    S = np.concatenate(
        [np.array([np.asarray(r["sums_out"], dtype=np.float64)[b, :nch[b]].sum()
                   for b in range(BS)])
         for r in res.results])                               # (B,)
    out = (expd / S[:, None]).astype(np.float32)
    return out.reshape(B, 1, L), res


def kernel(**inputs):
    out, _ = _run(inputs, trace=False)
    return out
